# revision 9
# baseline (speedup 1.0000x reference)
"""Trainium2 Bass kernel for nn_AttentionOperation (sparse_attention).

Computation (per the reference):
    sim  = QK^T                  [N,H,L,L]
    sim  = BN_heads(sim)         (stats over b,l,m per head)
    attn = softmax(sim, -1)
    rv   = attn @ V^T            [N,H,C,L] -> [N, H*C, L]
    rv   = BN_channels(rv)       (stats over b,l per channel)
    out  = gelu_exact(rv)

Sharding: one head per NeuronCore (H=8, n_cores=8).  Both BatchNorms are
then fully core-local (sim-BN stats are per head; val-BN channels
h*64..(h+1)*64-1 belong exactly to head h), so there is no communication.

Device-side math tricks:
  * BN1 mean/bias shift cancels inside the softmax, so only
    g = w_h * rsqrt(var + eps) is needed.  var comes from
    sum(sim) = sum_b ksum_b . qsum_b   and
    sum(sim^2) = sum_b tr((K Kt)(Q Qt))   -- tiny Gram matmuls instead of a
    second pass over the 4M-element sim matrix.
  * softmax denominator comes free from a ones-row appended to V^T.
  * rsqrt computed as exp(-0.5*ln(x+eps)) so ACT needs only the
    natural_log_exp table set (+ gelu at the end).

Everything is fp32.  The full per-core flow:
  grams -> g -> [QK^T -> exp(g*sim) -> attn@V] (pipelined) -> divide by
  denominator -> BN2 stats -> affine -> gelu -> DMA out.
"""

import numpy as np

N, H, D, L = 4, 8, 64, 1024
C = 64
NCH = L // 128  # m-chunks of 128
EPS = 1e-3
CNT = float(N * L * L)  # elements per head for sim BN stats

_CACHE = {}

# Feature toggles (fallbacks for ops that may not be supported end-to-end)
USE_PBCAST = True       # gpsimd.partition_broadcast vs PE ones-matmul bcast
RECIP_MODE = "approx"   # "approx" (custom DVE uop) or "lnexp" (ACT)


def _build_nc():
    import concourse.bacc as bacc
    import concourse.tile as tile
    import concourse.mybir as mybir

    f32 = mybir.dt.float32
    AF = mybir.ActivationFunctionType
    ALU = mybir.AluOpType

    nc = bacc.Bacc("TRN2", target_bir_lowering=False, debug=False)

    q2_d = nc.dram_tensor("q2", [128, 2, L], f32, kind="ExternalInput")
    k2_d = nc.dram_tensor("k2", [128, 2, L], f32, kind="ExternalInput")
    kto_d = nc.dram_tensor("kto", [128, N, NCH, 65], f32, kind="ExternalInput")
    qto_d = nc.dram_tensor("qto", [128, N, NCH, 65], f32, kind="ExternalInput")
    vo_d = nc.dram_tensor("vo", [128, N, NCH, 65], f32, kind="ExternalInput")
    ws_d = nc.dram_tensor("ws", [1, 1], f32, kind="ExternalInput")
    wv_d = nc.dram_tensor("wv", [64, 1], f32, kind="ExternalInput")
    bv_d = nc.dram_tensor("bv", [64, 1], f32, kind="ExternalInput")
    out_d = nc.dram_tensor("out", [N, 64, L], f32, kind="ExternalOutput")

    with tile.TileContext(nc) as tc:
        with (
            tc.tile_pool(name="cst", bufs=1) as cst,
            tc.tile_pool(name="sm", bufs=1) as sm,
            tc.tile_pool(name="exp", bufs=16) as epool,
            tc.tile_pool(name="rvp", bufs=4) as rvp,
            tc.tile_pool(name="outp", bufs=2) as outp,
            tc.tile_pool(name="ps", bufs=1, space="PSUM") as psp,
        ):
            # ---- input tiles + DMAs (transposed stat inputs first: they
            # gate g, which gates every exp) ----
            kto_sb = cst.tile([128, N, NCH, 65], f32)
            qto_sb = cst.tile([128, N, NCH, 65], f32)
            for b in range(N):
                nc.sync.dma_start(kto_sb[:, b], kto_d.ap()[:, b])
                nc.sync.dma_start(qto_sb[:, b], qto_d.ap()[:, b])

            q2_sb = cst.tile([128, 2, L], f32)
            k2_sb = cst.tile([128, 2, L], f32)
            for p in range(2):
                nc.sync.dma_start(q2_sb[:, p], q2_d.ap()[:, p])
                nc.sync.dma_start(k2_sb[:, p], k2_d.ap()[:, p])
            vo_sb = cst.tile([128, N, NCH, 65], f32)
            for b in range(N):
                nc.sync.dma_start(vo_sb[:, b], vo_d.ap()[:, b])

            ws_sb = cst.tile([1, 1], f32)
            nc.sync.dma_start(ws_sb[:], ws_d.ap())
            wv_sb = cst.tile([64, 1], f32)
            nc.sync.dma_start(wv_sb[:], wv_d.ap())
            bv_sb = cst.tile([64, 1], f32)
            nc.sync.dma_start(bv_sb[:], bv_d.ap())
            ones64 = cst.tile([64, 1], f32)
            nc.vector.memset(ones64[:], 1.0)
            ones128 = cst.tile([1, 128], f32)
            nc.vector.memset(ones128[:], 1.0)
            eps_sb = cst.tile([128, 1], f32)
            nc.vector.memset(eps_sb[:], EPS)

            # ---- BN1 stats via Gram matrices ----
            qparts = cst.tile([64, N], f32)
            sparts = cst.tile([64, N], f32)
            for b in range(N):
                gsb = {}
                for src, tag in ((kto_sb, "gk"), (qto_sb, "gq")):
                    gps = psp.tile([64, 65], f32, tag="gram", bufs=1,
                                   name=f"gram_ps_{tag}_{b}")
                    for c in range(NCH):
                        nc.tensor.matmul(
                            gps[:], src[:, b, c, 0:64], src[:, b, c, 0:65],
                            start=(c == 0), stop=(c == NCH - 1))
                    g_sb = sm.tile([64, 65], f32, tag=tag, bufs=2,
                                   name=f"gram_sb_{tag}_{b}")
                    nc.vector.tensor_copy(g_sb[:], gps[:])
                    gsb[tag] = g_sb
                pscr = sm.tile([64, 64], f32, tag="pscr", bufs=2,
                               name=f"pscr_{b}")
                nc.vector.tensor_tensor(
                    out=pscr[:], in0=gsb["gk"][:, 0:64], in1=gsb["gq"][:, 0:64],
                    op=ALU.mult)
                nc.vector.tensor_reduce(
                    out=qparts[:, b:b + 1], in_=pscr[:],
                    axis=mybir.AxisListType.X, op=ALU.add)
                nc.vector.tensor_tensor(
                    out=sparts[:, b:b + 1], in0=gsb["gk"][:, 64:65],
                    in1=gsb["gq"][:, 64:65], op=ALU.mult)

            qs = sm.tile([64, 2], f32, tag="qs", bufs=1)
            nc.vector.tensor_reduce(out=qs[:, 0:1], in_=qparts[:],
                                    axis=mybir.AxisListType.X, op=ALU.add)
            nc.vector.tensor_reduce(out=qs[:, 1:2], in_=sparts[:],
                                    axis=mybir.AxisListType.X, op=ALU.add)
            # partition-sum via PE: out [1,2] = [sum(sim^2), sum(sim)]
            scps = psp.tile([1, 2], f32, tag="gram", bufs=1)
            nc.tensor.matmul(scps[:], ones64[:], qs[:], start=True, stop=True)
            qs2 = sm.tile([1, 2], f32, tag="qs2", bufs=1)
            nc.vector.tensor_copy(qs2[:], scps[:])

            # var = E[x^2] - E[x]^2 ; g = w_h * rsqrt(var + eps)
            mean_t = sm.tile([1, 1], f32, tag="sc0", bufs=1)
            nc.vector.tensor_scalar_mul(mean_t[:], qs2[:, 1:2], 1.0 / CNT)
            eq_t = sm.tile([1, 1], f32, tag="sc1", bufs=1)
            nc.vector.tensor_scalar_mul(eq_t[:], qs2[:, 0:1], 1.0 / CNT)
            m2_t = sm.tile([1, 1], f32, tag="sc2", bufs=1)
            nc.vector.tensor_tensor(out=m2_t[:], in0=mean_t[:], in1=mean_t[:],
                                    op=ALU.mult)
            var_t = sm.tile([1, 1], f32, tag="sc3", bufs=1)
            nc.vector.tensor_tensor(out=var_t[:], in0=eq_t[:], in1=m2_t[:],
                                    op=ALU.subtract)
            ln_t = sm.tile([1, 1], f32, tag="sc4", bufs=1)
            nc.scalar.activation(ln_t[:], var_t[:], AF.Ln, bias=eps_sb[0:1])
            rs_t = sm.tile([1, 1], f32, tag="sc5", bufs=1)
            nc.scalar.activation(rs_t[:], ln_t[:], AF.Exp, scale=-0.5)
            g_t = sm.tile([1, 1], f32, tag="sc6", bufs=1)
            nc.vector.tensor_tensor(out=g_t[:], in0=rs_t[:], in1=ws_sb[:],
                                    op=ALU.mult)
            g128 = cst.tile([128, 1], f32)
            if USE_PBCAST:
                nc.gpsimd.partition_broadcast(g128[:], g_t[:], channels=128)
            else:
                gb_ps = psp.tile([128, 1], f32, tag="gram", bufs=1)
                nc.tensor.matmul(gb_ps[:], ones128[:], g_t[:],
                                 start=True, stop=True)
                nc.vector.tensor_copy(g128[:], gb_ps[:])

            # ---- main attention pipeline ----
            exp_tiles = [[None] * NCH for _ in range(N)]
            rv_tiles = []
            stats = cst.tile([64, 2 * N, 6], f32)

            for pair in range(2):
                for c in range(NCH):
                    for b_in in range(2):
                        b = 2 * pair + b_in
                        r0 = 64 * b_in
                        sim_ps = psp.tile([128, L], f32, tag="sim", bufs=2,
                                          name=f"sim_ps_{b}_{c}")
                        for half in range(2):
                            nc.tensor.matmul(
                                sim_ps[:, 512 * half:512 * (half + 1)],
                                k2_sb[r0:r0 + 64, pair, 128 * c:128 * (c + 1)],
                                q2_sb[r0:r0 + 64, pair, 512 * half:512 * (half + 1)],
                                start=True, stop=True)
                        ex = epool.tile([128, L], f32, tag="exp", bufs=16,
                                        name=f"exp_{b}_{c}")
                        nc.scalar.activation(ex[:], sim_ps[:], AF.Exp,
                                             scale=g128[:, 0:1])
                        exp_tiles[b][c] = ex

                for b_in in range(2):
                    b = 2 * pair + b_in
                    den_sb = sm.tile([1, L], f32, tag="den", bufs=2,
                                     name=f"den_{b}")
                    av_halves = []
                    for half in range(2):
                        av_ps = psp.tile([65, 512], f32, tag="av", bufs=3,
                                         name=f"av_ps_{b}_{half}")
                        for c in range(NCH):
                            nc.tensor.matmul(
                                av_ps[:], vo_sb[:, b, c, :],
                                exp_tiles[b][c][:, 512 * half:512 * (half + 1)],
                                start=(c == 0), stop=(c == NCH - 1))
                        nc.vector.tensor_copy(
                            den_sb[0:1, 512 * half:512 * (half + 1)],
                            av_ps[64:65, :])
                        av_halves.append(av_ps)
                    rcp_sb = sm.tile([1, L], f32, tag="rcp", bufs=2,
                                     name=f"rcp_{b}")
                    scr_sb = sm.tile([1, L], f32, tag="scr", bufs=2,
                                     name=f"scr_{b}")
                    if RECIP_MODE == "approx":
                        nc.vector.reciprocal_approx_accurate(
                            out=rcp_sb[:], in_=den_sb[:], scratch=scr_sb[:])
                    else:
                        nc.scalar.activation(scr_sb[:], den_sb[:], AF.Ln)
                        nc.scalar.activation(rcp_sb[:], scr_sb[:], AF.Exp,
                                             scale=-1.0)
                    rbc_sb = sm.tile([64, L], f32, tag="rbc", bufs=2,
                                     name=f"rbc_{b}")
                    if USE_PBCAST:
                        nc.gpsimd.partition_broadcast(rbc_sb[:], rcp_sb[:],
                                                      channels=64)
                    else:
                        for half in range(2):
                            rb_ps = psp.tile([64, 512], f32, tag="av", bufs=3,
                                             name=f"rb_ps_{b}_{half}")
                            nc.tensor.matmul(
                                rb_ps[:], ones128[:, 0:64],
                                rcp_sb[:, 512 * half:512 * (half + 1)],
                                start=True, stop=True)
                            nc.vector.tensor_copy(
                                rbc_sb[:, 512 * half:512 * (half + 1)],
                                rb_ps[:])
                    rv_sb = rvp.tile([64, L], f32, tag="rv", bufs=4,
                                     name=f"rv_{b}")
                    for half in range(2):
                        nc.vector.tensor_tensor(
                            out=rv_sb[:, 512 * half:512 * (half + 1)],
                            in0=av_halves[half][0:64, :],
                            in1=rbc_sb[:, 512 * half:512 * (half + 1)],
                            op=ALU.mult)
                        nc.vector.bn_stats(stats[:, 2 * b + half, :],
                                           rv_sb[:, 512 * half:512 * (half + 1)])
                    rv_tiles.append(rv_sb)

            # ---- BN2 + gelu epilogue ----
            mv = sm.tile([64, 2], f32, tag="mv", bufs=1)
            nc.vector.bn_aggr(mv[:], stats[:])
            lnv = sm.tile([64, 1], f32, tag="lnv", bufs=1)
            nc.scalar.activation(lnv[:], mv[:, 1:2], AF.Ln, bias=eps_sb[0:64])
            rsv = sm.tile([64, 1], f32, tag="rsv", bufs=1)
            nc.scalar.activation(rsv[:], lnv[:], AF.Exp, scale=-0.5)
            scale_c = sm.tile([64, 1], f32, tag="sclc", bufs=1)
            nc.vector.tensor_tensor(out=scale_c[:], in0=rsv[:], in1=wv_sb[:],
                                    op=ALU.mult)
            mt = sm.tile([64, 1], f32, tag="mt", bufs=1)
            nc.vector.tensor_tensor(out=mt[:], in0=mv[:, 0:1], in1=scale_c[:],
                                    op=ALU.mult)
            bias_c = sm.tile([64, 1], f32, tag="bsc", bufs=1)
            nc.vector.tensor_tensor(out=bias_c[:], in0=bv_sb[:], in1=mt[:],
                                    op=ALU.subtract)

            for b in range(N):
                aff = outp.tile([64, L], f32, tag="aff", bufs=2,
                                name=f"aff_{b}")
                nc.vector.tensor_scalar(
                    out=aff[:], in0=rv_tiles[b][:], scalar1=scale_c[:, 0:1],
                    scalar2=bias_c[:, 0:1], op0=ALU.mult, op1=ALU.add)
                osb = outp.tile([64, L], f32, tag="osb", bufs=2,
                                name=f"osb_{b}")
                nc.scalar.activation(osb[:], aff[:], AF.Gelu)
                nc.sync.dma_start(out_d.ap()[b], osb[:])

    nc.compile()
    return nc


def _host_inputs(query, key, value, bn_sim_weight, bn_sim_bias,
                 bn_val_weight, bn_val_bias, h):
    """Build the per-core (per-head) input map, with host-side layout prep."""
    f32 = np.float32
    qh = np.asarray(query[:, h], dtype=f32)   # [4, 64, 1024]
    kh = np.asarray(key[:, h], dtype=f32)
    vh = np.asarray(value[:, h], dtype=f32)

    def pack_pairs(x):
        # [4, 64, L] -> [128, 2, L]; row b_in*64+d, col (pair, l)
        return np.ascontiguousarray(
            x.reshape(2, 2, 64, L).transpose(1, 2, 0, 3).reshape(128, 2, L))

    def pack_t_ones(x):
        # [4, 64, L] -> [128(m), 4(b), 8(chunk), 65] with ones in col 64
        t = x.transpose(2, 0, 1).reshape(NCH, 128, N, 64).transpose(1, 2, 0, 3)
        out = np.empty((128, N, NCH, 65), dtype=f32)
        out[..., :64] = t
        out[..., 64] = 1.0
        return np.ascontiguousarray(out)

    return {
        "q2": pack_pairs(qh),
        "k2": pack_pairs(kh),
        "kto": pack_t_ones(kh),
        "qto": pack_t_ones(qh),
        "vo": pack_t_ones(vh),
        "ws": np.asarray(bn_sim_weight[h], dtype=f32).reshape(1, 1),
        "wv": np.ascontiguousarray(
            np.asarray(bn_val_weight[h * 64:(h + 1) * 64], dtype=f32)
            .reshape(64, 1)),
        "bv": np.ascontiguousarray(
            np.asarray(bn_val_bias[h * 64:(h + 1) * 64], dtype=f32)
            .reshape(64, 1)),
    }


def get_nc():
    if "nc" not in _CACHE:
        _CACHE["nc"] = _build_nc()
    return _CACHE["nc"]


def make_in_maps(**inputs):
    return [_host_inputs(
        inputs["query"], inputs["key"], inputs["value"],
        inputs["bn_sim_weight"], inputs["bn_sim_bias"],
        inputs["bn_val_weight"], inputs["bn_val_bias"], h) for h in range(H)]


def kernel(**inputs):
    from concourse.bass_utils import run_bass_kernel_spmd

    nc = get_nc()
    in_maps = make_in_maps(**inputs)
    res = run_bass_kernel_spmd(nc, in_maps, core_ids=list(range(H)))
    outs = [np.asarray(res.results[i]["out"]) for i in range(H)]
    return np.ascontiguousarray(
        np.concatenate(outs, axis=1).astype(np.float32))


# revision 18
# speedup vs baseline: 1.4899x; 1.4899x over previous
"""Trainium2 Bass kernel for nn_AttentionOperation (sparse_attention).

Computation (per the reference):
    sim  = QK^T                  [N,H,L,L]
    sim  = BN_heads(sim)         (stats over b,l,m per head)
    attn = softmax(sim, -1)
    rv   = attn @ V^T            [N,H,C,L] -> [N, H*C, L]
    rv   = BN_channels(rv)       (stats over b,l per channel)
    out  = gelu_exact(rv)

Sharding: one head per NeuronCore (H=8, n_cores=8).  Both BatchNorms are
then fully core-local (sim-BN stats are per head; val-BN channels
h*64..(h+1)*64-1 belong exactly to head h), so there is no communication.

Device-side math tricks:
  * BN1 mean/bias shift cancels inside the softmax, so only
    g = w_h * rsqrt(var + eps) is needed.  var comes from
    sum(sim) = sum_b ksum_b . qsum_b   and
    sum(sim^2) = sum_b tr((K Kt)(Q Qt))   -- tiny Gram matmuls instead of a
    second pass over the 4M-element sim matrix.
  * softmax denominator comes free from a ones-row appended to V^T.
  * rsqrt computed as exp(-0.5*ln(x+eps)) so ACT needs only the
    natural_log_exp table set (+ gelu at the end).

Everything is fp32.  The full per-core flow:
  grams -> g -> [QK^T -> exp(g*sim) -> attn@V] (pipelined) -> divide by
  denominator -> BN2 stats -> affine -> gelu -> DMA out.
"""

import numpy as np

N, H, D, L = 4, 8, 64, 1024
C = 64
NCH = L // 128  # m-chunks of 128
EPS = 1e-3
CNT = float(N * L * L)  # elements per head for sim BN stats

_CACHE = {}

# Feature toggles (fallbacks for ops that may not be supported end-to-end)
USE_PBCAST = True       # gpsimd.partition_broadcast vs PE ones-matmul bcast
RECIP_MODE = "approx"   # "approx" (custom DVE uop) or "lnexp" (ACT)


def _build_nc():
    import concourse.bacc as bacc
    import concourse.tile as tile
    import concourse.mybir as mybir

    f32 = mybir.dt.float32
    f16 = mybir.dt.float16
    AF = mybir.ActivationFunctionType
    ALU = mybir.AluOpType

    nc = bacc.Bacc("TRN2", target_bir_lowering=False, debug=False)

    q2_d = nc.dram_tensor("q2", [128, 2, L], f16, kind="ExternalInput")
    k2_d = nc.dram_tensor("k2", [128, 2, L], f16, kind="ExternalInput")
    kto_d = nc.dram_tensor("kto", [128, N, NCH, 65], f16, kind="ExternalInput")
    qto_d = nc.dram_tensor("qto", [128, N, NCH, 65], f16, kind="ExternalInput")
    vo_d = nc.dram_tensor("vo", [128, N, NCH, 65], f16, kind="ExternalInput")
    ws_d = nc.dram_tensor("ws", [1, 1], f32, kind="ExternalInput")
    wv_d = nc.dram_tensor("wv", [64, 1], f32, kind="ExternalInput")
    bv_d = nc.dram_tensor("bv", [64, 1], f32, kind="ExternalInput")
    out_d = nc.dram_tensor("out", [N, 64, L], f32, kind="ExternalOutput")

    with tile.TileContext(nc) as tc:
        with (
            tc.tile_pool(name="cst", bufs=1) as cst,
            tc.tile_pool(name="sm", bufs=1) as sm,
            tc.tile_pool(name="exp", bufs=16) as epool,
            tc.tile_pool(name="rvp", bufs=4) as rvp,
            tc.tile_pool(name="outp", bufs=2) as outp,
            tc.tile_pool(name="ps", bufs=1, space="PSUM") as psp,
        ):
            # ---- input tiles + DMAs (transposed stat inputs first: they
            # gate g, which gates every exp) ----
            kto_sb = cst.tile([128, N, NCH, 65], f16)
            qto_sb = cst.tile([128, N, NCH, 65], f16)
            for b in range(N):
                nc.sync.dma_start(kto_sb[:, b], kto_d.ap()[:, b])
                nc.sync.dma_start(qto_sb[:, b], qto_d.ap()[:, b])

            q2_sb = cst.tile([128, 2, L], f16)
            k2_sb = cst.tile([128, 2, L], f16)
            for p in range(2):
                nc.sync.dma_start(q2_sb[:, p], q2_d.ap()[:, p])
                nc.sync.dma_start(k2_sb[:, p], k2_d.ap()[:, p])
            vo_sb = cst.tile([128, N, NCH, 65], f16)
            for b in range(N):
                nc.sync.dma_start(vo_sb[:, b], vo_d.ap()[:, b])

            ws_sb = cst.tile([1, 1], f32)
            nc.sync.dma_start(ws_sb[:], ws_d.ap())
            wv_sb = cst.tile([64, 1], f32)
            nc.sync.dma_start(wv_sb[:], wv_d.ap())
            bv_sb = cst.tile([64, 1], f32)
            nc.sync.dma_start(bv_sb[:], bv_d.ap())
            ones128 = cst.tile([1, 128], f32)
            nc.vector.memset(ones128[:], 1.0)
            eps_sb = cst.tile([128, 1], f32)
            nc.vector.memset(eps_sb[:], EPS)

            # ---- BN1 stats via Gram matrices ----
            ones64 = cst.tile([64, 1], f32)
            nc.vector.memset(ones64[:], 1.0)
            qparts = cst.tile([64, N], f32)
            sparts = cst.tile([64, N], f32)
            for b in range(N):
                gsb = {}
                for src, tag in ((kto_sb, "gk"), (qto_sb, "gq")):
                    gps = psp.tile([64, 65], f32, tag="gram", bufs=2,
                                   name=f"gram_ps_{tag}_{b}")
                    for c in range(NCH):
                        nc.tensor.matmul(
                            gps[:], src[:, b, c, 0:64], src[:, b, c, 0:65],
                            start=(c == 0), stop=(c == NCH - 1))
                    g_sb = sm.tile([64, 65], f32, tag=tag, bufs=2,
                                   name=f"gram_sb_{tag}_{b}")
                    nc.vector.tensor_copy(g_sb[:], gps[:])
                    gsb[tag] = g_sb
                pscr = sm.tile([64, 64], f32, tag="pscr", bufs=2,
                               name=f"pscr_{b}")
                nc.vector.tensor_tensor(
                    out=pscr[:], in0=gsb["gk"][:, 0:64], in1=gsb["gq"][:, 0:64],
                    op=ALU.mult)
                nc.vector.tensor_reduce(
                    out=qparts[:, b:b + 1], in_=pscr[:],
                    axis=mybir.AxisListType.X, op=ALU.add)
                nc.vector.tensor_tensor(
                    out=sparts[:, b:b + 1], in0=gsb["gk"][:, 64:65],
                    in1=gsb["gq"][:, 64:65], op=ALU.mult)

            qsp = sm.tile([64, 2], f32, tag="qs", bufs=1)
            nc.vector.tensor_reduce(out=qsp[:, 0:1], in_=qparts[:],
                                    axis=mybir.AxisListType.X, op=ALU.add)
            nc.vector.tensor_reduce(out=qsp[:, 1:2], in_=sparts[:],
                                    axis=mybir.AxisListType.X, op=ALU.add)
            # partition-sum via PE: out [1,2] = [sum(sim^2), sum(sim)]
            scps = psp.tile([1, 2], f32, tag="gram", bufs=2)
            nc.tensor.matmul(scps[:], ones64[:], qsp[:], start=True,
                             stop=True)
            qs2 = sm.tile([1, 2], f32, tag="qs2", bufs=1)
            nc.vector.tensor_copy(qs2[:], scps[:])

            # DVE-only rsqrt(x + eps): quake seed + 3 Newton iterations.
            # Keeps ScalarE's table needs down to {Exp, Gelu} (2 loads).
            i32 = mybir.dt.int32

            def dve_rsqrt(dst_ap, x_ap, p, pref):
                xe = sm.tile([p, 1], f32, tag=f"{pref}xe", bufs=1,
                             name=f"{pref}_xe")
                nc.vector.tensor_scalar_add(xe[:], x_ap, EPS)
                sh = sm.tile([p, 1], i32, tag=f"{pref}sh", bufs=1,
                             name=f"{pref}_sh")
                nc.vector.tensor_scalar(
                    out=sh[:], in0=xe[:].bitcast(i32), scalar1=1,
                    scalar2=None, op0=ALU.arith_shift_right)
                magic = sm.tile([p, 1], i32, tag=f"{pref}mg", bufs=1,
                                name=f"{pref}_mg")
                nc.vector.memset(magic[:], 0x5F3759DF)
                y = sm.tile([p, 1], f32, tag=f"{pref}y", bufs=1,
                            name=f"{pref}_y")
                nc.vector.tensor_tensor(out=y[:].bitcast(i32), in0=magic[:],
                                        in1=sh[:], op=ALU.subtract)
                t = sm.tile([p, 1], f32, tag=f"{pref}t", bufs=1,
                            name=f"{pref}_t")
                for _ in range(3):
                    nc.vector.tensor_tensor(out=t[:], in0=xe[:], in1=y[:],
                                            op=ALU.mult)
                    nc.vector.tensor_tensor(out=t[:], in0=t[:], in1=y[:],
                                            op=ALU.mult)
                    nc.vector.tensor_scalar(out=t[:], in0=t[:], scalar1=-0.5,
                                            scalar2=1.5, op0=ALU.mult,
                                            op1=ALU.add)
                    nc.vector.tensor_tensor(out=y[:], in0=y[:], in1=t[:],
                                            op=ALU.mult)
                nc.vector.tensor_copy(dst_ap, y[:])

            # var = E[x^2] - E[x]^2 ; g = w_h * rsqrt(var + eps)
            mean_t = sm.tile([1, 1], f32, tag="sc0", bufs=1)
            nc.vector.tensor_scalar_mul(mean_t[:], qs2[:, 1:2], 1.0 / CNT)
            eq_t = sm.tile([1, 1], f32, tag="sc1", bufs=1)
            nc.vector.tensor_scalar_mul(eq_t[:], qs2[:, 0:1], 1.0 / CNT)
            m2_t = sm.tile([1, 1], f32, tag="sc2", bufs=1)
            nc.vector.tensor_tensor(out=m2_t[:], in0=mean_t[:], in1=mean_t[:],
                                    op=ALU.mult)
            var_t = sm.tile([1, 1], f32, tag="sc3", bufs=1)
            nc.vector.tensor_tensor(out=var_t[:], in0=eq_t[:], in1=m2_t[:],
                                    op=ALU.subtract)
            rs_t = sm.tile([1, 1], f32, tag="sc5", bufs=1)
            dve_rsqrt(rs_t[:], var_t[:], 1, "g")
            g_t = sm.tile([1, 1], f32, tag="sc6", bufs=1)
            nc.vector.tensor_tensor(out=g_t[:], in0=rs_t[:], in1=ws_sb[:],
                                    op=ALU.mult)
            g128 = cst.tile([128, 1], f32)
            if USE_PBCAST:
                nc.gpsimd.partition_broadcast(g128[:], g_t[:], channels=128)
            else:
                gb_ps = psp.tile([128, 1], f32, tag="gram", bufs=1)
                nc.tensor.matmul(gb_ps[:], ones128[:], g_t[:],
                                 start=True, stop=True)
                nc.vector.tensor_copy(g128[:], gb_ps[:])

            # ---- main attention pipeline ----
            exp_tiles = [[None] * NCH for _ in range(N)]
            rv_tiles = []
            stats = cst.tile([64, 2 * N, 6], f32)

            for pair in range(2):
                for c in range(NCH):
                    for b_in in range(2):
                        b = 2 * pair + b_in
                        r0 = 64 * b_in
                        sim_ps = psp.tile([128, L], f32, tag="sim", bufs=2,
                                          name=f"sim_ps_{b}_{c}")
                        for half in range(2):
                            nc.tensor.matmul(
                                sim_ps[:, 512 * half:512 * (half + 1)],
                                k2_sb[r0:r0 + 64, pair, 128 * c:128 * (c + 1)],
                                q2_sb[r0:r0 + 64, pair, 512 * half:512 * (half + 1)],
                                start=True, stop=True)
                        ex = epool.tile([128, L], f16, tag="exp", bufs=16,
                                        name=f"exp_{b}_{c}")
                        nc.scalar.activation(ex[:], sim_ps[:], AF.Exp,
                                             scale=g128[:, 0:1])
                        exp_tiles[b][c] = ex

                for b_in in range(2):
                    b = 2 * pair + b_in
                    den_sb = sm.tile([1, L], f32, tag="den", bufs=2,
                                     name=f"den_{b}")
                    av_halves = []
                    for half in range(2):
                        av_ps = psp.tile([65, 512], f32, tag="av", bufs=2,
                                         name=f"av_ps_{b}_{half}")
                        for c in range(NCH):
                            nc.tensor.matmul(
                                av_ps[:], vo_sb[:, b, c, :],
                                exp_tiles[b][c][:, 512 * half:512 * (half + 1)],
                                start=(c == 0), stop=(c == NCH - 1))
                        nc.vector.tensor_copy(
                            den_sb[0:1, 512 * half:512 * (half + 1)],
                            av_ps[64:65, :])
                        av_halves.append(av_ps)
                    rcp_sb = sm.tile([1, L], f32, tag="rcp", bufs=2,
                                     name=f"rcp_{b}")
                    scr_sb = sm.tile([1, L], f32, tag="scr", bufs=2,
                                     name=f"scr_{b}")
                    if RECIP_MODE == "approx":
                        nc.vector.reciprocal_approx_accurate(
                            out=rcp_sb[:], in_=den_sb[:], scratch=scr_sb[:])
                    else:
                        nc.scalar.activation(scr_sb[:], den_sb[:], AF.Ln)
                        nc.scalar.activation(rcp_sb[:], scr_sb[:], AF.Exp,
                                             scale=-1.0)
                    rbc_sb = sm.tile([64, L], f32, tag="rbc", bufs=2,
                                     name=f"rbc_{b}")
                    if USE_PBCAST:
                        nc.gpsimd.partition_broadcast(rbc_sb[:], rcp_sb[:],
                                                      channels=64)
                    else:
                        for half in range(2):
                            rb_ps = psp.tile([64, 512], f32, tag="av", bufs=2,
                                             name=f"rb_ps_{b}_{half}")
                            nc.tensor.matmul(
                                rb_ps[:], ones128[:, 0:64],
                                rcp_sb[:, 512 * half:512 * (half + 1)],
                                start=True, stop=True)
                            nc.vector.tensor_copy(
                                rbc_sb[:, 512 * half:512 * (half + 1)],
                                rb_ps[:])
                    rv_sb = rvp.tile([64, L], f32, tag="rv", bufs=4,
                                     name=f"rv_{b}")
                    for half in range(2):
                        nc.vector.tensor_tensor(
                            out=rv_sb[:, 512 * half:512 * (half + 1)],
                            in0=av_halves[half][0:64, :],
                            in1=rbc_sb[:, 512 * half:512 * (half + 1)],
                            op=ALU.mult)
                        nc.vector.bn_stats(stats[:, 2 * b + half, :],
                                           rv_sb[:, 512 * half:512 * (half + 1)])
                    rv_tiles.append(rv_sb)

            # ---- BN2 + gelu epilogue ----
            mv = sm.tile([64, 2], f32, tag="mv", bufs=1)
            nc.vector.bn_aggr(mv[:], stats[:])
            rsv = sm.tile([64, 1], f32, tag="rsv", bufs=1)
            dve_rsqrt(rsv[:], mv[:, 1:2], 64, "v")
            scale_c = sm.tile([64, 1], f32, tag="sclc", bufs=1)
            nc.vector.tensor_tensor(out=scale_c[:], in0=rsv[:], in1=wv_sb[:],
                                    op=ALU.mult)
            mt = sm.tile([64, 1], f32, tag="mt", bufs=1)
            nc.vector.tensor_tensor(out=mt[:], in0=mv[:, 0:1], in1=scale_c[:],
                                    op=ALU.mult)
            bias_c = sm.tile([64, 1], f32, tag="bsc", bufs=1)
            nc.vector.tensor_tensor(out=bias_c[:], in0=bv_sb[:], in1=mt[:],
                                    op=ALU.subtract)

            for b in range(N):
                aff = outp.tile([64, L], f32, tag="aff", bufs=2,
                                name=f"aff_{b}")
                nc.vector.tensor_scalar(
                    out=aff[:], in0=rv_tiles[b][:], scalar1=scale_c[:, 0:1],
                    scalar2=bias_c[:, 0:1], op0=ALU.mult, op1=ALU.add)
                osb = outp.tile([64, L], f32, tag="osb", bufs=2,
                                name=f"osb_{b}")
                nc.scalar.activation(osb[:], aff[:], AF.Gelu)
                nc.sync.dma_start(out_d.ap()[b], osb[:])

    nc.compile()
    return nc


def _host_inputs(query, key, value, bn_sim_weight, bn_sim_bias,
                 bn_val_weight, bn_val_bias, h):
    """Build the per-core (per-head) input map, with host-side layout prep."""
    f32 = np.float32
    qh = np.asarray(query[:, h], dtype=f32)   # [4, 64, 1024]
    kh = np.asarray(key[:, h], dtype=f32)
    vh = np.asarray(value[:, h], dtype=f32)

    def pack_pairs(x):
        # [4, 64, L] -> [128, 2, L]; row b_in*64+d, col (pair, l)
        return np.ascontiguousarray(
            x.reshape(2, 2, 64, L).transpose(1, 2, 0, 3).reshape(128, 2, L)
            .astype(np.float16))

    def pack_t_ones(x):
        # [4, 64, L] -> [128(m), 4(b), 8(chunk), 65] with ones in col 64
        t = x.transpose(2, 0, 1).reshape(NCH, 128, N, 64).transpose(1, 2, 0, 3)
        out = np.empty((128, N, NCH, 65), dtype=np.float16)
        out[..., :64] = t.astype(np.float16)
        out[..., 64] = 1.0
        return np.ascontiguousarray(out)

    return {
        "q2": pack_pairs(qh),
        "k2": pack_pairs(kh),
        "kto": pack_t_ones(kh),
        "qto": pack_t_ones(qh),
        "vo": pack_t_ones(vh),
        "ws": np.asarray(bn_sim_weight[h], dtype=f32).reshape(1, 1),
        "wv": np.ascontiguousarray(
            np.asarray(bn_val_weight[h * 64:(h + 1) * 64], dtype=f32)
            .reshape(64, 1)),
        "bv": np.ascontiguousarray(
            np.asarray(bn_val_bias[h * 64:(h + 1) * 64], dtype=f32)
            .reshape(64, 1)),
    }


def get_nc():
    if "nc" not in _CACHE:
        _CACHE["nc"] = _build_nc()
    return _CACHE["nc"]


def make_in_maps(**inputs):
    return [_host_inputs(
        inputs["query"], inputs["key"], inputs["value"],
        inputs["bn_sim_weight"], inputs["bn_sim_bias"],
        inputs["bn_val_weight"], inputs["bn_val_bias"], h) for h in range(H)]


def kernel(**inputs):
    from concourse.bass_utils import run_bass_kernel_spmd

    nc = get_nc()
    in_maps = make_in_maps(**inputs)
    res = run_bass_kernel_spmd(nc, in_maps, core_ids=list(range(H)))
    outs = [np.asarray(res.results[i]["out"]) for i in range(H)]
    return np.ascontiguousarray(
        np.concatenate(outs, axis=1).astype(np.float32))


# revision 26
# speedup vs baseline: 1.5959x; 1.0712x over previous
"""Trainium2 Bass kernel for nn_AttentionOperation (sparse_attention).

Computation (per the reference):
    sim  = QK^T                  [N,H,L,L]
    sim  = BN_heads(sim)         (stats over b,l,m per head)
    attn = softmax(sim, -1)
    rv   = attn @ V^T            [N,H,C,L] -> [N, H*C, L]
    rv   = BN_channels(rv)       (stats over b,l per channel)
    out  = gelu_exact(rv)

Sharding: one head per NeuronCore (H=8, n_cores=8).  Both BatchNorms are
then fully core-local (sim-BN stats are per head; val-BN channels
h*64..(h+1)*64-1 belong exactly to head h), so there is no communication.

Device-side tricks:
  * BN1 mean/bias shift cancels inside the softmax, so only
    g = w_h * rsqrt(var + eps) is needed.  var comes from tiny Gram
    matmuls: sum(sim^2) = sum_b tr((K Kt)(Q Qt)), sum(sim) = sum_b
    ksum_b . qsum_b.  K and Q chunks are stacked into one [m,129]
    operand so one matmul per (batch, m-chunk) yields KK, QQ, ksum,
    qsum together; the QQ block is realigned with a SBUF->SBUF DMA.
  * matmul operands are fp16 (fp32 matmul = 2 half-rate passes on PE).
    PSUM accumulation stays fp32; per-element error ~2^-11 averages out
    in the BN statistics.
  * softmax denominator comes free from a ones-row appended to V^T.
  * rsqrt is a DVE-only quake-seed Newton iteration, and 1/den uses the
    custom-DVE reciprocal_approx_fast, so ScalarE needs only the Exp
    and Gelu table sets (2 loads total).
  * BN2 affine is folded into the Gelu activation's scale/bias operands.
"""

import numpy as np

N, H, D, L = 4, 8, 64, 1024
C = 64
NCH = L // 128  # m-chunks of 128
EPS = 1e-3
CNT = float(N * L * L)  # elements per head for sim BN stats

_CACHE = {}


def _build_nc():
    import concourse.bacc as bacc
    import concourse.tile as tile
    import concourse.mybir as mybir

    f32 = mybir.dt.float32
    f16 = mybir.dt.float16
    i32 = mybir.dt.int32
    AF = mybir.ActivationFunctionType
    ALU = mybir.AluOpType

    nc = bacc.Bacc("TRN2", target_bir_lowering=False, debug=False)

    q2_d = nc.dram_tensor("q2", [128, 2, L], f16, kind="ExternalInput")
    k2_d = nc.dram_tensor("k2", [128, 2, L], f16, kind="ExternalInput")
    kqo_d = nc.dram_tensor("kqo", [128, N, NCH, 129], f16,
                           kind="ExternalInput")
    vo_d = nc.dram_tensor("vo", [128, N, NCH, 65], f16, kind="ExternalInput")
    ws_d = nc.dram_tensor("ws", [1, 1], f32, kind="ExternalInput")
    wv_d = nc.dram_tensor("wv", [64, 1], f32, kind="ExternalInput")
    bv_d = nc.dram_tensor("bv", [64, 1], f32, kind="ExternalInput")
    out_d = nc.dram_tensor("out", [N, 64, L], f32, kind="ExternalOutput")

    with tile.TileContext(nc) as tc:
        with (
            tc.tile_pool(name="cst", bufs=1) as cst,
            tc.tile_pool(name="sm", bufs=1) as sm,
            tc.tile_pool(name="exp", bufs=16) as epool,
            tc.tile_pool(name="rvp", bufs=4) as rvp,
            tc.tile_pool(name="outp", bufs=2) as outp,
            tc.tile_pool(name="ps", bufs=1, space="PSUM") as psp,
        ):
            # ---- input DMAs (gram inputs first: they gate g -> every exp)
            kqo_sb = cst.tile([128, N, NCH, 129], f16)
            for b in range(N):
                nc.sync.dma_start(kqo_sb[:, b], kqo_d.ap()[:, b])
            q2_sb = cst.tile([128, 2, L], f16)
            k2_sb = cst.tile([128, 2, L], f16)
            for p in range(2):
                nc.sync.dma_start(q2_sb[:, p], q2_d.ap()[:, p])
                nc.sync.dma_start(k2_sb[:, p], k2_d.ap()[:, p])
            vo_sb = cst.tile([128, N, NCH, 65], f16)
            for b in range(N):
                nc.sync.dma_start(vo_sb[:, b], vo_d.ap()[:, b])

            ws_sb = cst.tile([1, 1], f32)
            nc.sync.dma_start(ws_sb[:], ws_d.ap())
            wv_sb = cst.tile([64, 1], f32)
            nc.sync.dma_start(wv_sb[:], wv_d.ap())
            bv_sb = cst.tile([64, 1], f32)
            nc.sync.dma_start(bv_sb[:], bv_d.ap())
            ones128 = cst.tile([1, 128], f32)
            nc.vector.memset(ones128[:], 1.0)
            ones64 = cst.tile([64, 1], f32)
            nc.vector.memset(ones64[:], 1.0)

            # ---- BN1 stats: one stacked gram matmul per (batch, chunk) ----
            # G[b] = [k|q|1]^T [k|q|1]:  KK = G[0:64,0:64],
            # QQ = G[64:128,64:128], ksum = G[0:64,128], qsum = G[64:128,128]
            parts = cst.tile([64, 2, N], f32)
            qparts = parts[:, 0, :]
            sparts = parts[:, 1, :]
            for b in range(N):
                gps = psp.tile([128, 129], f32, tag="av", bufs=2,
                               name=f"gram_ps_{b}")
                for c in range(NCH):
                    nc.tensor.matmul(
                        gps[:], kqo_sb[:, b, c, 0:128], kqo_sb[:, b, c, :],
                        start=(c == 0), stop=(c == NCH - 1))
                g_sb = sm.tile([128, 129], f32, tag="gk", bufs=2,
                               name=f"gram_sb_{b}")
                nc.vector.tensor_copy(g_sb[:], gps[:])
                # realign the QQ block onto partitions 0-63
                qq_sb = sm.tile([64, 65], f32, tag="gq", bufs=2,
                                name=f"qq_sb_{b}")
                nc.sync.dma_start(qq_sb[:], g_sb[64:128, 64:129])
                pscr = sm.tile([64, 64], f32, tag="pscr", bufs=2,
                               name=f"pscr_{b}")
                nc.vector.tensor_tensor(
                    out=pscr[:], in0=g_sb[0:64, 0:64], in1=qq_sb[:, 0:64],
                    op=ALU.mult)
                nc.vector.tensor_reduce(
                    out=qparts[:, b:b + 1], in_=pscr[:],
                    axis=mybir.AxisListType.X, op=ALU.add)
                nc.vector.tensor_tensor(
                    out=sparts[:, b:b + 1], in0=g_sb[0:64, 128:129],
                    in1=qq_sb[:, 64:65], op=ALU.mult)

            qsp = sm.tile([64, 2], f32, tag="qs", bufs=1)
            nc.vector.tensor_reduce(out=qsp[:], in_=parts[:],
                                    axis=mybir.AxisListType.X, op=ALU.add)
            # partition-sum via PE: out [1,2] = [sum(sim^2), sum(sim)]
            scps = psp.tile([1, 2], f32, tag="av", bufs=2)
            nc.tensor.matmul(scps[:], ones64[:], qsp[:], start=True,
                             stop=True)
            qs2 = sm.tile([1, 2], f32, tag="qs2", bufs=1)
            nc.vector.tensor_copy(qs2[:], scps[:])

            # DVE-only rsqrt(x + eps): quake seed + 3 Newton iterations.
            def dve_rsqrt(dst_ap, x_ap, p, pref):
                xe = sm.tile([p, 1], f32, tag=f"{pref}xe", bufs=1,
                             name=f"{pref}_xe")
                nc.vector.tensor_scalar_add(xe[:], x_ap, EPS)
                sh = sm.tile([p, 1], i32, tag=f"{pref}sh", bufs=1,
                             name=f"{pref}_sh")
                nc.vector.tensor_scalar(
                    out=sh[:], in0=xe[:].bitcast(i32), scalar1=1,
                    scalar2=None, op0=ALU.arith_shift_right)
                magic = sm.tile([p, 1], i32, tag=f"{pref}mg", bufs=1,
                                name=f"{pref}_mg")
                nc.vector.memset(magic[:], 0x5F3759DF)
                y = sm.tile([p, 1], f32, tag=f"{pref}y", bufs=1,
                            name=f"{pref}_y")
                nc.vector.tensor_tensor(out=y[:].bitcast(i32), in0=magic[:],
                                        in1=sh[:], op=ALU.subtract)
                t = sm.tile([p, 1], f32, tag=f"{pref}t", bufs=1,
                            name=f"{pref}_t")
                n_it = 2  # seed err 3.4% -> 1.7e-3 -> 4e-6: plenty here
                for it in range(n_it):
                    nc.vector.tensor_tensor(out=t[:], in0=y[:], in1=y[:],
                                            op=ALU.mult)
                    nc.vector.scalar_tensor_tensor(
                        out=t[:], in0=t[:], scalar=-0.5, in1=xe[:],
                        op0=ALU.mult, op1=ALU.mult)
                    nc.vector.scalar_tensor_tensor(
                        out=(dst_ap if it == n_it - 1 else y[:]), in0=t[:],
                        scalar=1.5, in1=y[:], op0=ALU.add, op1=ALU.mult)

            # var = E[x^2] - E[x]^2 ; g = w_h * rsqrt(var + eps)
            eq_t = sm.tile([1, 1], f32, tag="sc1", bufs=1)
            nc.vector.tensor_scalar_mul(eq_t[:], qs2[:, 0:1], 1.0 / CNT)
            m2_t = sm.tile([1, 1], f32, tag="sc2", bufs=1)
            nc.vector.scalar_tensor_tensor(
                out=m2_t[:], in0=qs2[:, 1:2], scalar=1.0 / (CNT * CNT),
                in1=qs2[:, 1:2], op0=ALU.mult, op1=ALU.mult)
            var_t = sm.tile([1, 1], f32, tag="sc3", bufs=1)
            nc.vector.tensor_tensor(out=var_t[:], in0=eq_t[:], in1=m2_t[:],
                                    op=ALU.subtract)
            rs_t = sm.tile([1, 1], f32, tag="sc5", bufs=1)
            dve_rsqrt(rs_t[:], var_t[:], 1, "g")
            g_t = sm.tile([1, 1], f32, tag="sc6", bufs=1)
            nc.vector.tensor_tensor(out=g_t[:], in0=rs_t[:], in1=ws_sb[:],
                                    op=ALU.mult)
            # broadcast g to all 128 partitions via a tiny PE outer product
            gb_ps = psp.tile([128, 1], f32, tag="av", bufs=2)
            nc.tensor.matmul(gb_ps[:], ones128[:], g_t[:], start=True,
                             stop=True)
            g128 = cst.tile([128, 1], f32)
            nc.vector.tensor_copy(g128[:], gb_ps[:])

            # ---- main attention pipeline ----
            exp_tiles = [[None] * NCH for _ in range(N)]
            rv_tiles = []
            stats = cst.tile([64, 2 * N, 6], f32)

            for pair in range(2):
                for c in range(NCH):
                    for b_in in range(2):
                        b = 2 * pair + b_in
                        r0 = 64 * b_in
                        sim_ps = psp.tile([128, L], f32, tag="sim", bufs=3,
                                          name=f"sim_ps_{b}_{c}")
                        for half in range(2):
                            nc.tensor.matmul(
                                sim_ps[:, 512 * half:512 * (half + 1)],
                                k2_sb[r0:r0 + 64, pair, 128 * c:128 * (c + 1)],
                                q2_sb[r0:r0 + 64, pair, 512 * half:512 * (half + 1)],
                                start=True, stop=True)
                        ex = epool.tile([128, L], f16, tag="exp", bufs=16,
                                        name=f"exp_{b}_{c}")
                        nc.scalar.activation(ex[:], sim_ps[:], AF.Exp,
                                             scale=g128[:, 0:1])
                        exp_tiles[b][c] = ex

                for b_in in range(2):
                    b = 2 * pair + b_in
                    den_sb = sm.tile([1, L], f32, tag="den", bufs=2,
                                     name=f"den_{b}")
                    rcp_sb = sm.tile([1, L], f32, tag="rcp", bufs=2,
                                     name=f"rcp_{b}")
                    av_halves = []
                    for half in range(2):
                        av_ps = psp.tile([65, 512], f32, tag="av", bufs=2,
                                         name=f"av_ps_{b}_{half}")
                        for c in range(NCH):
                            nc.tensor.matmul(
                                av_ps[:], vo_sb[:, b, c, :],
                                exp_tiles[b][c][:, 512 * half:512 * (half + 1)],
                                start=(c == 0), stop=(c == NCH - 1))
                        nc.vector.tensor_copy(
                            den_sb[0:1, 512 * half:512 * (half + 1)],
                            av_ps[64:65, :])
                        av_halves.append(av_ps)
                    nc.vector.reciprocal_approx_fast(
                        out=rcp_sb[:], in_=den_sb[:])
                    rbc_sb = sm.tile([64, L], f32, tag="rbc", bufs=2,
                                     name=f"rbc_{b}")
                    nc.gpsimd.partition_broadcast(rbc_sb[:], rcp_sb[:],
                                                  channels=64)
                    rv_sb = rvp.tile([64, L], f32, tag="rv", bufs=4,
                                     name=f"rv_{b}")
                    for half in range(2):
                        nc.vector.tensor_tensor(
                            out=rv_sb[:, 512 * half:512 * (half + 1)],
                            in0=av_halves[half][0:64, :],
                            in1=rbc_sb[:, 512 * half:512 * (half + 1)],
                            op=ALU.mult)
                        nc.vector.bn_stats(stats[:, 2 * b + half, :],
                                           rv_sb[:, 512 * half:512 * (half + 1)])
                    rv_tiles.append(rv_sb)

            # ---- BN2 + gelu epilogue (affine folded into Gelu) ----
            mv = sm.tile([64, 2], f32, tag="mv", bufs=1)
            nc.vector.bn_aggr(mv[:], stats[:])
            rsv = sm.tile([64, 1], f32, tag="rsv", bufs=1)
            dve_rsqrt(rsv[:], mv[:, 1:2], 64, "v")
            scale_c = sm.tile([64, 1], f32, tag="sclc", bufs=1)
            nc.vector.tensor_tensor(out=scale_c[:], in0=rsv[:], in1=wv_sb[:],
                                    op=ALU.mult)
            mt = sm.tile([64, 1], f32, tag="mt", bufs=1)
            nc.vector.tensor_tensor(out=mt[:], in0=mv[:, 0:1], in1=scale_c[:],
                                    op=ALU.mult)
            bias_c = sm.tile([64, 1], f32, tag="bsc", bufs=1)
            nc.vector.tensor_tensor(out=bias_c[:], in0=bv_sb[:], in1=mt[:],
                                    op=ALU.subtract)

            for b in range(N):
                osb = outp.tile([64, L], f32, tag="osb", bufs=2,
                                name=f"osb_{b}")
                nc.scalar.activation(osb[:], rv_tiles[b][:], AF.Gelu,
                                     bias=bias_c[:, 0:1],
                                     scale=scale_c[:, 0:1])
                nc.sync.dma_start(out_d.ap()[b], osb[:])

    nc.compile()
    return nc


def _host_inputs(query, key, value, bn_sim_weight, bn_sim_bias,
                 bn_val_weight, bn_val_bias, h):
    """Build the per-core (per-head) input map, with host-side layout prep."""
    f32 = np.float32
    f16 = np.float16
    qh = np.asarray(query[:, h], dtype=f32)   # [4, 64, 1024]
    kh = np.asarray(key[:, h], dtype=f32)
    vh = np.asarray(value[:, h], dtype=f32)

    def pack_pairs(x):
        # [4, 64, L] -> [128, 2, L]; row b_in*64+d, col (pair, l)
        return np.ascontiguousarray(
            x.reshape(2, 2, 64, L).transpose(1, 2, 0, 3).reshape(128, 2, L)
            .astype(f16))

    def chunked_t(x):
        # [4, 64, L] -> [128(m), 4(b), 8(chunk), 64]
        return x.transpose(2, 0, 1).reshape(NCH, 128, N, 64).transpose(
            1, 2, 0, 3)

    kq = np.empty((128, N, NCH, 129), dtype=f16)
    kq[..., 0:64] = chunked_t(kh).astype(f16)
    kq[..., 64:128] = chunked_t(qh).astype(f16)
    kq[..., 128] = 1.0

    vo = np.empty((128, N, NCH, 65), dtype=f16)
    vo[..., :64] = chunked_t(vh).astype(f16)
    vo[..., 64] = 1.0

    return {
        "q2": pack_pairs(qh),
        "k2": pack_pairs(kh),
        "kqo": np.ascontiguousarray(kq),
        "vo": np.ascontiguousarray(vo),
        "ws": np.asarray(bn_sim_weight[h], dtype=f32).reshape(1, 1),
        "wv": np.ascontiguousarray(
            np.asarray(bn_val_weight[h * 64:(h + 1) * 64], dtype=f32)
            .reshape(64, 1)),
        "bv": np.ascontiguousarray(
            np.asarray(bn_val_bias[h * 64:(h + 1) * 64], dtype=f32)
            .reshape(64, 1)),
    }


def get_nc():
    if "nc" not in _CACHE:
        _CACHE["nc"] = _build_nc()
    return _CACHE["nc"]


def make_in_maps(**inputs):
    return [_host_inputs(
        inputs["query"], inputs["key"], inputs["value"],
        inputs["bn_sim_weight"], inputs["bn_sim_bias"],
        inputs["bn_val_weight"], inputs["bn_val_bias"], h) for h in range(H)]


def kernel(**inputs):
    from concourse.bass_utils import run_bass_kernel_spmd

    nc = get_nc()
    in_maps = make_in_maps(**inputs)
    res = run_bass_kernel_spmd(nc, in_maps, core_ids=list(range(H)))
    outs = [np.asarray(res.results[i]["out"]) for i in range(H)]
    return np.ascontiguousarray(
        np.concatenate(outs, axis=1).astype(np.float32))
